# revision 25
# baseline (speedup 1.0000x reference)
"""NT-Xent contrastive loss on 8 Trainium2 NeuronCores (symmetric scheme).

Reference (B=4096, D=128, T=0.5):
    z = row-normalize(concat(emb_i, emb_j))           # [8192, 128]
    sim = z @ z.T
    S_r = sum_l exp(sim[r,l]/T),  denom_r = S_r - e^2
    loss = mean_r ( log(denom_r) ) - mean_r(pos_r)/T

Exploits sim's symmetry: each exp(sim[r,l]/T) for r != l is computed ONCE
and credited to BOTH row r (row-sum) and row l (column-sum).  16 row-blocks
of 512; core c owns blocks c and c+8, processing 17 column-chunks of 512
(block A: wrap-offsets 0..8, block B: 0..7) — 4.46M exp elements per core
instead of 8.39M.  Off-diagonal chunks at offsets 1..7 are computed once and
credited to the partner block via a column-sum; the {c, c+8} pair chunk is
computed only by block A (column-sum credits block B); diagonal chunks
contribute row-sums and the constant e^2 is removed on the host.  Per-row
coverage: A rows 9+7, B rows 8+7+1 = all 16 chunks exactly once.

The host pre-normalizes, transposes, casts to bf16 and ROTATES columns by
512c per core, so the SPMD program is core-uniform: zT [128d, 8192cols]
where col j maps to original row (512c + j) mod 8192.  Engine split:
  PE     gram matmuls bf16 (3 x [128,512] per PSUM tile) + one ones-matmul
         per column-sum chunk (kept off the gram critical path)
  ACT    exp, PSUM fp32 -> SBUF bf16, [128,1536] instructions with
         accum_out giving fp32 row-sum partials — THE critical path
         (~38us busy; everything else hides under it)
  DVE    4->1 row-tile folds (2x-mode bf16 adds) feeding the column-sum
         matmuls, PSUM->SBUF copies of column-sums, final reduce
  DMA    zT loads as 28 contiguous panels, triggers round-robin on
         sync/gpsimd/scalar (one engine's DGE trigger costs ~0.7us each)
Host combines partials in fp64: S_r, denom, log, positives, mean.

Measured: 61.4us on HW (baseline 131.9us).  Span anatomy: ~7us NEFF entry
(fixed), ~8us DMA/matmul ramp, ~38us saturated exp phase, ~2us column-sum
straggler, ~11us fixed teardown (semaphore barrier storm — also present in
the baseline; emitted by the toolchain, not this kernel's IR).
"""

import math

import numpy as np

import concourse.bass as bass
import concourse.mybir as mybir
import concourse.tile as tile
from concourse.bass_utils import run_bass_kernel_spmd

B = 4096
D = 128
NR = 2 * B               # 8192 rows
N_CORES = 8
P = 128
NBLK = 16                # row blocks of 512
BLK = 512
CH = 512                 # col chunk
NCOLS = NR               # all 8192 cols of zT visible per core
TEMPERATURE = 0.5
INV_T = 1.0 / TEMPERATURE
E2 = math.exp(INV_T)     # exp(sim_rr / T), sim_rr == 1

# Core c owns row-blocks c (A) and c+8 (B) of 16.  With columns rotated by
# 512c, block A sits at rot chunk 0 and computes chunks at offsets 0..8
# (column-sums for 1..8 — its offset-8 chunk is the {c, c+8} pair, computed
# only here, so block B's rows receive it as a column-sum); block B sits at
# rot chunk 8 and computes offsets 0..7 (column-sums 1..7).  Per-row
# coverage: A rows 9 own + 7 credits, B rows 8 own + 7 + 1 credits = 16.
# Group layout per 3-bank PSUM/ACT tile; no-column-sum offsets (0=diag) last.
BLOCK_GROUPS = [
    [(1, 2, 3), (4, 5, 6), (7, 8, 0)],
    [(1, 2, 3), (4, 5, 6), (7, 0)],
]
BLOCK_CS = [list(range(1, 9)), list(range(1, 8))]
BLOCK_SLOTS = [[k for g in grps for k in g] for grps in BLOCK_GROUPS]
BLOCK_COL0 = [0, 8 * CH]
NCS = sum(len(c) for c in BLOCK_CS)   # 15 column-sum chunks per core
CS_SLOT = {}
for _bl in range(2):
    for _kk in BLOCK_CS[_bl]:
        CS_SLOT[(_bl, _kk)] = len([1 for b2 in range(_bl) for _ in BLOCK_CS[b2]]) + BLOCK_CS[_bl].index(_kk)


def _kk_slot(bl: int, kk: int) -> int:
    """Free-dim slot of chunk-offset kk inside the per-(block,t) E row."""
    return BLOCK_SLOTS[bl].index(kk)


_NC = None
TRACE = False            # test.py flips this for profiled runs
_LAST_RESULT = None      # test.py reads exec_time_ns / trace from here

f32 = mybir.dt.float32
bf16 = mybir.dt.bfloat16
f8e4 = mybir.dt.float8e4
AF = mybir.ActivationFunctionType
OP = mybir.AluOpType
DR = mybir.MatmulPerfMode.DoubleRow


def _patched_clear_and_free_semaphores(self, sems):
    """Replacement for Bass.clear_and_free_semaphores: the stock version
    emits a raw-ISA EVENT_SEMAPHORE_RANGE_CLEAR that this toolchain's walrus
    rejects ("ISA wrong length").  Emit BIR-native per-sem `wr-imm 0`
    updates on gpsimd NOPs instead."""
    if not sems:
        return
    sem_nums = [s.num if hasattr(s, "num") else s for s in sems]
    for n in sem_nums:
        inst = self.gpsimd.nop()
        upd = mybir.SyncUpdate(
            sync_type="semaphore",
            id=n,
            update_mode="sem-wr-imm",
            update_value=0,
            ant_name=f"semclr{n}",
        )
        si = inst.ins.sync_info
        if si is None:
            inst.ins.sync_info = mybir.SyncInfo(on_wait=[], on_update=[upd])
        else:
            si.on_update.append(upd)
    self._state.prepend_free_semaphores(sem_nums)
    for poison_set in self._tile_sem_poison_stack:
        poison_set.update(sem_nums)


def _hoist_excess_waits(nc):
    """This toolchain's walrus allows only ONE sync-wait on most compute
    instruction structs; Tile sometimes attaches two.  Hoist all-but-one wait
    onto same-engine EventSemaphore carriers inserted immediately before."""
    n = 0
    for f in nc.m.functions:
        for blk in f.blocks:
            out = []
            for inst in blk.instructions:
                si = inst.sync_info
                tn = type(inst).__name__
                if (
                    si is not None
                    and len(si.on_wait) > 1
                    and tn != "InstEventSemaphore"
                ):
                    waits = list(si.on_wait)
                    keep, extra = waits[-1:], waits[:-1]
                    while extra:
                        grp, extra = extra[:2], extra[2:]
                        es = mybir.InstEventSemaphore(
                            name=f"wcarrier_{n}", ins=[], outs=[]
                        )
                        n += 1
                        es.engine = inst.engine
                        es.sync_info = mybir.SyncInfo(on_wait=list(grp), on_update=[])
                        out.append(es)
                    inst.sync_info = mybir.SyncInfo(
                        on_wait=keep, on_update=list(si.on_update)
                    )
                out.append(inst)
            blk.instructions[:] = out


def _build_nc(for_sim: bool = False) -> bass.Bass:
    """for_sim=True skips the walrus workarounds (_hoist_excess_waits and the
    patched semaphore clear) — CoreSim's race detector can't digest them (the
    stock baseline kernel trips the same assertion), and they only matter for
    the HW toolchain."""
    nc = bass.Bass("TRN2", target_bir_lowering=False, debug=False)
    import types as _types

    if not for_sim:
        nc.clear_and_free_semaphores = _types.MethodType(
            _patched_clear_and_free_semaphores, nc
        )

    # host supplies zT as 16 small [128,128] panels (cols 0..2047, needed
    # first) followed by 12 big [128,512] panels (cols 2048..8191)
    zta_d = nc.dram_tensor("zta", [16, P, P], bf16, kind="ExternalInput")
    ztb_d = nc.dram_tensor("ztb", [12, P, 4 * P], bf16, kind="ExternalInput")
    rs_d = nc.dram_tensor("rs", [P, 8], f32, kind="ExternalOutput")
    cs_d = nc.dram_tensor("cs", [NCS, CH], f32, kind="ExternalOutput")
    # sub-diagonal column-sums: per block, tiles tj=0..2 contribute widths
    # 384/256/128 at offsets 0/384/640 (the diagonal chunk is computed as an
    # upper-triangle of 128-row tiles; the lower half is credited back here)
    csd_d = nc.dram_tensor("csd", [2, 768], f32, kind="ExternalOutput")

    with tile.TileContext(nc) as tc:
        with (
            tc.tile_pool(name="singles", bufs=1) as singles,
            tc.tile_pool(name="scratch", bufs=2) as scratch,
            tc.tile_pool(name="psum_mm", bufs=2, space="PSUM") as psum_mm,
            tc.tile_pool(name="psum_cs", bufs=2, space="PSUM") as psum_cs,
        ):
            zt = singles.tile([P, NCOLS], bf16, tag="zt")

            # ---- load zT panels FIRST; trigger DMAs round-robin on two
            # engines (a single engine's DGE trigger costs ~0.6-0.8us each
            # and would serialize the prologue) ----
            trig = [nc.sync, nc.gpsimd, nc.scalar]
            for i in range(16):
                trig[i % 3].dma_start(
                    out=zt[:, i * P : (i + 1) * P], in_=zta_d.ap()[i]
                )
            for i in range(12):
                trig[(16 + i) % 3].dma_start(
                    out=zt[:, 2048 + i * 4 * P : 2048 + (i + 1) * 4 * P],
                    in_=ztb_d.ap()[i],
                )

            ones = singles.tile([P, 1], bf16, tag="ones")
            nc.vector.memset(ones[:], 1.0)

            # E[bl]: [128, t, slot, col] bf16 exp values for one block
            # (consumed by the column-sum path; row sums come from the
            # activation accumulator in fp32)
            eb = [
                singles.tile(
                    [P, 4, len(BLOCK_SLOTS[bl]), CH], bf16,
                    tag=f"eb{bl}", name=f"eb{bl}",
                )
                for bl in range(2)
            ]
            esums = singles.tile([P, 8, 3], f32, tag="esums")
            rs_sb = singles.tile([P, 8], f32, tag="rs_sb")
            cs_sb = singles.tile([P, NCS, CH], f32, tag="cs_sb")
            csd_sb = singles.tile([P, 2, 768], f32, tag="csd_sb")

            # preload the Exp activation table while DMAs run
            warm = singles.tile([P, 1], f32, tag="warm")
            nc.vector.memset(warm[:], 0.0)
            nc.scalar.activation(warm[:], warm[:], AF.Exp)

            def emit_colsum(bl: int, kk: int):
                """4 row-tiles of E[bl] chunk kk -> one [1,512] column-sum.
                DVE folds 4 row-tiles to 1 (2x-mode bf16 adds), PE does a
                single ones-matmul so it stays off the gram critical path."""
                sl = _kk_slot(bl, kk)
                s2 = scratch.tile([P, 2, CH], bf16, tag="s2", name=f"s2_{bl}_{kk}")
                nc.vector.tensor_tensor(
                    s2[:], eb[bl][:, 0:2, sl], eb[bl][:, 2:4, sl], OP.add
                )
                s4 = scratch.tile([P, CH], bf16, tag="s4", name=f"s4_{bl}_{kk}")
                nc.vector.tensor_tensor(s4[:], s2[:, 0], s2[:, 1], OP.add)
                csp = psum_cs.tile([P, CH], f32, tag="csp", name=f"csp_{bl}_{kk}")
                nc.tensor.matmul(csp[0:1, :], ones[:], s4[:], start=True, stop=True)
                slot = CS_SLOT[(bl, kk)]
                nc.vector.tensor_copy(cs_sb[0:1, slot], csp[0:1, :])

            for bl in range(2):
                col0 = BLOCK_COL0[bl]
                for gi, grp in enumerate(BLOCK_GROUPS[bl]):
                    g0 = sum(len(g) for g in BLOCK_GROUPS[bl][:gi])
                    for t in range(4):
                        lh = slice(col0 + P * t, col0 + P * (t + 1))
                        pg = psum_mm.tile(
                            [P, 3 * CH], f32, tag="pg", name=f"pg{bl}_{gi}_{t}"
                        )
                        width = 0
                        for kj, kk in enumerate(grp):
                            if kk == 0:
                                # diagonal chunk: only cols >= own tile start
                                # (upper triangle of 128-row tiles)
                                w = CH - P * t
                                rh = slice(col0 + P * t, col0 + CH)
                            else:
                                w = CH
                                rh = slice(col0 + CH * kk, col0 + CH * (kk + 1))
                            nc.tensor.matmul(
                                pg[:, width : width + w],
                                zt[:, lh],
                                zt[:, rh],
                                start=True,
                                stop=True,
                            )
                            width += w
                        ebflat = eb[bl][:].rearrange("p t s c -> p t (s c)")
                        nc.scalar.activation(
                            ebflat[:, t, g0 * CH : g0 * CH + width],
                            pg[:, 0:width],
                            AF.Exp,
                            scale=INV_T,
                            accum_out=esums[:, 4 * bl + t, gi : gi + 1],
                        )
                    # column-sum chunks that become ready after this group
                    # (they need all 4 row-tiles); defer the final group's
                    # to after the block so only one chunk tails the kernel.
                    if gi < len(BLOCK_GROUPS[bl]) - 1:
                        for kk in grp:
                            if kk in BLOCK_CS[bl]:
                                emit_colsum(bl, kk)
                # tail column-sums for this block (last group's)
                for kk in BLOCK_GROUPS[bl][-1]:
                    if kk in BLOCK_CS[bl]:
                        emit_colsum(bl, kk)
                # sub-diagonal credits: colsum tile tj's diag-chunk cols
                # [128(tj+1), 512) — E offsets [128, 512-128*tj)
                sl0 = _kk_slot(bl, 0)
                off = 0
                for tj in range(3):
                    w2 = CH - P * (tj + 1)
                    cspd = psum_cs.tile(
                        [P, CH], f32, tag="csp", name=f"cspd_{bl}_{tj}"
                    )
                    nc.tensor.matmul(
                        cspd[0:1, 0:w2],
                        ones[:],
                        eb[bl][:, tj, sl0, P : P + w2],
                        start=True,
                        stop=True,
                    )
                    nc.vector.tensor_copy(
                        csd_sb[0:1, bl, off : off + w2], cspd[0:1, 0:w2]
                    )
                    off += w2

            # fold the 3 per-(bl,t) accumulator partials into rs [128, 8]
            nc.vector.tensor_reduce(
                rs_sb[:].rearrange("p (a b) -> p a b", b=1),
                esums[:],
                axis=mybir.AxisListType.X,
                op=OP.add,
            )

            nc.sync.dma_start(out=cs_d.ap()[0 : NCS - 1], in_=cs_sb[0:1, 0 : NCS - 1, :])
            nc.sync.dma_start(out=csd_d.ap(), in_=csd_sb[0:1, :, :])
            nc.sync.dma_start(out=rs_d.ap(), in_=rs_sb[:])
            nc.sync.dma_start(out=cs_d.ap()[NCS - 1 : NCS], in_=cs_sb[0:1, NCS - 1 : NCS, :])

    if not for_sim:
        _hoist_excess_waits(nc)
    return nc


def _get_nc() -> bass.Bass:
    global _NC
    if _NC is None:
        _NC = _build_nc()
    return _NC


def _host_inputs(emb_i: np.ndarray, emb_j: np.ndarray):
    """Normalize, transpose, bf16-cast, and build per-core rotated panels."""
    import ml_dtypes

    reps = np.concatenate(
        [np.asarray(emb_i, np.float32), np.asarray(emb_j, np.float32)], axis=0
    )
    z = reps / np.linalg.norm(reps, axis=1, keepdims=True)
    zt = np.ascontiguousarray(z.T.astype(ml_dtypes.bfloat16))  # [128, 8192]
    in_maps = []
    for c in range(N_CORES):
        lo = BLK * c
        cols = (lo + np.arange(NCOLS)) % NR
        ztc = zt[:, cols]                                   # [128, 8192]
        za = np.ascontiguousarray(
            ztc[:, :2048].reshape(P, 16, P).transpose(1, 0, 2)
        )                                                    # [16, 128, 128]
        zb = np.ascontiguousarray(
            ztc[:, 2048:].reshape(P, 12, 4 * P).transpose(1, 0, 2)
        )                                                    # [12, 128, 512]
        in_maps.append({"zta": za, "ztb": zb})
    return z, in_maps


def kernel(emb_i: np.ndarray, emb_j: np.ndarray) -> np.ndarray:
    global _LAST_RESULT
    z, in_maps = _host_inputs(emb_i, emb_j)

    kw = {}
    if TRACE:
        import os
        import tempfile

        kw["tmpdir"] = tempfile.mkdtemp(prefix="trace_", dir=os.getcwd())
    res = run_bass_kernel_spmd(
        _get_nc(), in_maps, list(range(N_CORES)), trace=TRACE, **kw
    )
    _LAST_RESULT = res

    # ---- host combine (fp64) ----
    S = np.zeros(NR, dtype=np.float64)
    for c in range(N_CORES):
        rs = np.asarray(res.results[c]["rs"], np.float64)    # [128, 8]
        cs = np.asarray(res.results[c]["cs"], np.float64)    # [15, 512]
        csd = np.asarray(res.results[c]["csd"], np.float64)  # [2, 768]
        base = BLK * c
        for bl in range(2):
            col0 = BLOCK_COL0[bl]
            for t in range(4):
                rows = (base + col0 + P * t + np.arange(P)) % NR
                S[rows] += rs[:, 4 * bl + t]
            for kk in BLOCK_CS[bl]:
                cols = (base + col0 + CH * kk + np.arange(CH)) % NR
                S[cols] += cs[CS_SLOT[(bl, kk)]]
            off = 0
            for tj in range(3):
                w2 = CH - P * (tj + 1)
                cols = (base + col0 + P * (tj + 1) + np.arange(w2)) % NR
                S[cols] += csd[bl, off : off + w2]
                off += w2

    denom = S - E2
    partner = (np.arange(NR) + B) % NR
    pos = np.einsum(
        "ij,ij->i", z.astype(np.float64), z[partner].astype(np.float64)
    )
    loss = np.mean(np.log(denom)) - INV_T * np.mean(pos)
    return np.asarray(np.float32(loss))


# revision 26
# speedup vs baseline: 1.0646x; 1.0646x over previous
"""NT-Xent contrastive loss on 8 Trainium2 NeuronCores (symmetric scheme).

Reference (B=4096, D=128, T=0.5):
    z = row-normalize(concat(emb_i, emb_j))           # [8192, 128]
    sim = z @ z.T
    S_r = sum_l exp(sim[r,l]/T),  denom_r = S_r - e^2
    loss = mean_r ( log(denom_r) ) - mean_r(pos_r)/T

Exploits sim's symmetry: each exp(sim[r,l]/T) for r != l is computed ONCE
and credited to BOTH row r (row-sum) and row l (column-sum).  16 row-blocks
of 512; core c owns blocks c and c+8, processing 17 column-chunks of 512
(block A: wrap-offsets 0..8, block B: 0..7) — 4.46M exp elements per core
instead of 8.39M.  Off-diagonal chunks at offsets 1..7 are computed once and
credited to the partner block via a column-sum; the {c, c+8} pair chunk is
computed only by block A (column-sum credits block B); diagonal chunks
contribute row-sums and the constant e^2 is removed on the host.  Per-row
coverage: A rows 9+7, B rows 8+7+1 = all 16 chunks exactly once.

The host pre-normalizes, transposes, casts to bf16 and ROTATES columns by
512c per core, so the SPMD program is core-uniform: zT [128d, 8192cols]
where col j maps to original row (512c + j) mod 8192.  Engine split:
  PE     gram matmuls bf16 (3 x [128,512] per PSUM tile) + one ones-matmul
         per column-sum chunk (kept off the gram critical path)
  ACT    exp, PSUM fp32 -> SBUF bf16, [128,1536] instructions with
         accum_out giving fp32 row-sum partials — THE critical path
         (~38us busy; everything else hides under it)
  DVE    4->1 row-tile folds (2x-mode bf16 adds) feeding the column-sum
         matmuls, PSUM->SBUF copies of column-sums, final reduce
  DMA    zT loads as 28 contiguous panels, triggers round-robin on
         sync/gpsimd/scalar (one engine's DGE trigger costs ~0.7us each)
Host combines partials in fp64: S_r, denom, log, positives, mean.

Measured: 61.4us on HW (baseline 131.9us).  Span anatomy: ~7us NEFF entry
(fixed), ~8us DMA/matmul ramp, ~38us saturated exp phase, ~2us column-sum
straggler, ~11us fixed teardown (semaphore barrier storm — also present in
the baseline; emitted by the toolchain, not this kernel's IR).
"""

import math

import numpy as np

import concourse.bass as bass
import concourse.mybir as mybir
import concourse.tile as tile
from concourse.bass_utils import run_bass_kernel_spmd

B = 4096
D = 128
NR = 2 * B               # 8192 rows
N_CORES = 8
P = 128
NBLK = 16                # row blocks of 512
BLK = 512
CH = 512                 # col chunk
NCOLS = NR               # all 8192 cols of zT visible per core
TEMPERATURE = 0.5
INV_T = 1.0 / TEMPERATURE
E2 = math.exp(INV_T)     # exp(sim_rr / T), sim_rr == 1

# Core c owns row-blocks c (A) and c+8 (B) of 16.  With columns rotated by
# 512c, block A sits at rot chunk 0 and computes chunks at offsets 0..8
# (column-sums for 1..8 — its offset-8 chunk is the {c, c+8} pair, computed
# only here, so block B's rows receive it as a column-sum); block B sits at
# rot chunk 8 and computes offsets 0..7 (column-sums 1..7).  Per-row
# coverage: A rows 9 own + 7 credits, B rows 8 own + 7 + 1 credits = 16.
# Group layout per 3-bank PSUM/ACT tile; no-column-sum offsets (0=diag) last.
BLOCK_GROUPS = [
    [(1, 2, 3), (4, 5, 6), (7, 8, 0)],
    [(1, 2, 3), (4, 5, 6), (7, 0)],
]
BLOCK_CS = [list(range(1, 9)), list(range(1, 8))]
BLOCK_SLOTS = [[k for g in grps for k in g] for grps in BLOCK_GROUPS]
BLOCK_COL0 = [0, 8 * CH]
NCS = sum(len(c) for c in BLOCK_CS)   # 15 column-sum chunks per core
CS_SLOT = {}
for _bl in range(2):
    for _kk in BLOCK_CS[_bl]:
        CS_SLOT[(_bl, _kk)] = len([1 for b2 in range(_bl) for _ in BLOCK_CS[b2]]) + BLOCK_CS[_bl].index(_kk)


def _kk_slot(bl: int, kk: int) -> int:
    """Free-dim slot of chunk-offset kk inside the per-(block,t) E row."""
    return BLOCK_SLOTS[bl].index(kk)


_NC = None
TRACE = False            # test.py flips this for profiled runs
_LAST_RESULT = None      # test.py reads exec_time_ns / trace from here

f32 = mybir.dt.float32
bf16 = mybir.dt.bfloat16
f8e4 = mybir.dt.float8e4
AF = mybir.ActivationFunctionType
OP = mybir.AluOpType
DR = mybir.MatmulPerfMode.DoubleRow


def _patched_clear_and_free_semaphores(self, sems):
    """Replacement for Bass.clear_and_free_semaphores: the stock version
    emits a raw-ISA EVENT_SEMAPHORE_RANGE_CLEAR that this toolchain's walrus
    rejects ("ISA wrong length").  Emit BIR-native per-sem `wr-imm 0`
    updates on gpsimd NOPs instead."""
    if not sems:
        return
    sem_nums = [s.num if hasattr(s, "num") else s for s in sems]
    for n in sem_nums:
        inst = self.gpsimd.nop()
        upd = mybir.SyncUpdate(
            sync_type="semaphore",
            id=n,
            update_mode="sem-wr-imm",
            update_value=0,
            ant_name=f"semclr{n}",
        )
        si = inst.ins.sync_info
        if si is None:
            inst.ins.sync_info = mybir.SyncInfo(on_wait=[], on_update=[upd])
        else:
            si.on_update.append(upd)
    self._state.prepend_free_semaphores(sem_nums)
    for poison_set in self._tile_sem_poison_stack:
        poison_set.update(sem_nums)


def _hoist_excess_waits(nc):
    """This toolchain's walrus allows only ONE sync-wait on most compute
    instruction structs; Tile sometimes attaches two.  Hoist all-but-one wait
    onto same-engine EventSemaphore carriers inserted immediately before."""
    n = 0
    for f in nc.m.functions:
        for blk in f.blocks:
            out = []
            for inst in blk.instructions:
                si = inst.sync_info
                tn = type(inst).__name__
                if (
                    si is not None
                    and len(si.on_wait) > 1
                    and tn != "InstEventSemaphore"
                ):
                    waits = list(si.on_wait)
                    keep, extra = waits[-1:], waits[:-1]
                    while extra:
                        grp, extra = extra[:2], extra[2:]
                        es = mybir.InstEventSemaphore(
                            name=f"wcarrier_{n}", ins=[], outs=[]
                        )
                        n += 1
                        es.engine = inst.engine
                        es.sync_info = mybir.SyncInfo(on_wait=list(grp), on_update=[])
                        out.append(es)
                    inst.sync_info = mybir.SyncInfo(
                        on_wait=keep, on_update=list(si.on_update)
                    )
                out.append(inst)
            blk.instructions[:] = out


def _build_nc(for_sim: bool = False) -> bass.Bass:
    """for_sim=True skips the walrus workarounds (_hoist_excess_waits and the
    patched semaphore clear) — CoreSim's race detector can't digest them (the
    stock baseline kernel trips the same assertion), and they only matter for
    the HW toolchain."""
    nc = bass.Bass("TRN2", target_bir_lowering=False, debug=False)
    import types as _types

    if not for_sim:
        nc.clear_and_free_semaphores = _types.MethodType(
            _patched_clear_and_free_semaphores, nc
        )

    # host supplies zT as 16 small [128,128] panels (cols 0..2047, needed
    # first) followed by 12 big [128,512] panels (cols 2048..8191)
    zta_d = nc.dram_tensor("zta", [16, P, P], bf16, kind="ExternalInput")
    ztb_d = nc.dram_tensor("ztb", [12, P, 4 * P], bf16, kind="ExternalInput")
    rs_d = nc.dram_tensor("rs", [P, 8], f32, kind="ExternalOutput")
    cs_d = nc.dram_tensor("cs", [NCS, CH], f32, kind="ExternalOutput")

    with tile.TileContext(nc) as tc:
        with (
            tc.tile_pool(name="singles", bufs=1) as singles,
            tc.tile_pool(name="scratch", bufs=2) as scratch,
            tc.tile_pool(name="psum_mm", bufs=2, space="PSUM") as psum_mm,
            tc.tile_pool(name="psum_cs", bufs=2, space="PSUM") as psum_cs,
        ):
            zt = singles.tile([P, NCOLS], bf16, tag="zt")

            # ---- load zT panels FIRST; trigger DMAs round-robin on two
            # engines (a single engine's DGE trigger costs ~0.6-0.8us each
            # and would serialize the prologue) ----
            trig = [nc.sync, nc.gpsimd, nc.scalar]
            for i in range(16):
                trig[i % 3].dma_start(
                    out=zt[:, i * P : (i + 1) * P], in_=zta_d.ap()[i]
                )
            for i in range(12):
                trig[(16 + i) % 3].dma_start(
                    out=zt[:, 2048 + i * 4 * P : 2048 + (i + 1) * 4 * P],
                    in_=ztb_d.ap()[i],
                )

            ones = singles.tile([P, 1], bf16, tag="ones")
            nc.vector.memset(ones[:], 1.0)

            # E[bl]: [128, t, slot, col] bf16 exp values for one block
            # (consumed by the column-sum path; row sums come from the
            # activation accumulator in fp32)
            eb = [
                singles.tile(
                    [P, 4, len(BLOCK_SLOTS[bl]), CH], bf16,
                    tag=f"eb{bl}", name=f"eb{bl}",
                )
                for bl in range(2)
            ]
            esums = singles.tile([P, 8, 3], f32, tag="esums")
            rs_sb = singles.tile([P, 8], f32, tag="rs_sb")
            cs_sb = singles.tile([P, NCS, CH], f32, tag="cs_sb")

            # preload the Exp activation table while DMAs run
            warm = singles.tile([P, 1], f32, tag="warm")
            nc.vector.memset(warm[:], 0.0)
            nc.scalar.activation(warm[:], warm[:], AF.Exp)

            def emit_colsum(bl: int, kk: int):
                """4 row-tiles of E[bl] chunk kk -> one [1,512] column-sum.
                DVE folds 4 row-tiles to 1 (2x-mode bf16 adds), PE does a
                single ones-matmul so it stays off the gram critical path."""
                sl = _kk_slot(bl, kk)
                s2 = scratch.tile([P, 2, CH], bf16, tag="s2", name=f"s2_{bl}_{kk}")
                nc.vector.tensor_tensor(
                    s2[:], eb[bl][:, 0:2, sl], eb[bl][:, 2:4, sl], OP.add
                )
                s4 = scratch.tile([P, CH], bf16, tag="s4", name=f"s4_{bl}_{kk}")
                nc.vector.tensor_tensor(s4[:], s2[:, 0], s2[:, 1], OP.add)
                csp = psum_cs.tile([P, CH], f32, tag="csp", name=f"csp_{bl}_{kk}")
                nc.tensor.matmul(csp[0:1, :], ones[:], s4[:], start=True, stop=True)
                slot = CS_SLOT[(bl, kk)]
                nc.vector.tensor_copy(cs_sb[0:1, slot], csp[0:1, :])

            for bl in range(2):
                col0 = BLOCK_COL0[bl]
                for gi, grp in enumerate(BLOCK_GROUPS[bl]):
                    g0 = sum(len(g) for g in BLOCK_GROUPS[bl][:gi])
                    for t in range(4):
                        lh = slice(col0 + P * t, col0 + P * (t + 1))
                        pg = psum_mm.tile(
                            [P, 3 * CH], f32, tag="pg", name=f"pg{bl}_{gi}_{t}"
                        )
                        for kj, kk in enumerate(grp):
                            rh = slice(col0 + CH * kk, col0 + CH * (kk + 1))
                            nc.tensor.matmul(
                                pg[:, kj * CH : (kj + 1) * CH],
                                zt[:, lh],
                                zt[:, rh],
                                start=True,
                                stop=True,
                            )
                        nc.scalar.activation(
                            eb[bl][:, t, g0 : g0 + len(grp)],
                            pg[:, 0 : len(grp) * CH],
                            AF.Exp,
                            scale=INV_T,
                            accum_out=esums[:, 4 * bl + t, gi : gi + 1],
                        )
                    # column-sum chunks that become ready after this group
                    # (they need all 4 row-tiles); defer the final group's
                    # to after the block so only one chunk tails the kernel.
                    if gi < len(BLOCK_GROUPS[bl]) - 1:
                        for kk in grp:
                            if kk in BLOCK_CS[bl]:
                                emit_colsum(bl, kk)
                # tail column-sums for this block (last group's)
                for kk in BLOCK_GROUPS[bl][-1]:
                    if kk in BLOCK_CS[bl]:
                        emit_colsum(bl, kk)

            # fold the 3 per-(bl,t) accumulator partials into rs [128, 8]
            nc.vector.tensor_reduce(
                rs_sb[:].rearrange("p (a b) -> p a b", b=1),
                esums[:],
                axis=mybir.AxisListType.X,
                op=OP.add,
            )

            nc.sync.dma_start(out=cs_d.ap()[0 : NCS - 1], in_=cs_sb[0:1, 0 : NCS - 1, :])
            nc.sync.dma_start(out=rs_d.ap(), in_=rs_sb[:])
            nc.sync.dma_start(out=cs_d.ap()[NCS - 1 : NCS], in_=cs_sb[0:1, NCS - 1 : NCS, :])

    if not for_sim:
        _hoist_excess_waits(nc)
    return nc


def _get_nc() -> bass.Bass:
    global _NC
    if _NC is None:
        _NC = _build_nc()
    return _NC


def _host_inputs(emb_i: np.ndarray, emb_j: np.ndarray):
    """Normalize, transpose, bf16-cast, and build per-core rotated panels."""
    import ml_dtypes

    reps = np.concatenate(
        [np.asarray(emb_i, np.float32), np.asarray(emb_j, np.float32)], axis=0
    )
    z = reps / np.linalg.norm(reps, axis=1, keepdims=True)
    zt = np.ascontiguousarray(z.T.astype(ml_dtypes.bfloat16))  # [128, 8192]
    in_maps = []
    for c in range(N_CORES):
        lo = BLK * c
        cols = (lo + np.arange(NCOLS)) % NR
        ztc = zt[:, cols]                                   # [128, 8192]
        za = np.ascontiguousarray(
            ztc[:, :2048].reshape(P, 16, P).transpose(1, 0, 2)
        )                                                    # [16, 128, 128]
        zb = np.ascontiguousarray(
            ztc[:, 2048:].reshape(P, 12, 4 * P).transpose(1, 0, 2)
        )                                                    # [12, 128, 512]
        in_maps.append({"zta": za, "ztb": zb})
    return z, in_maps


def kernel(emb_i: np.ndarray, emb_j: np.ndarray) -> np.ndarray:
    global _LAST_RESULT
    z, in_maps = _host_inputs(emb_i, emb_j)

    kw = {}
    if TRACE:
        import os
        import tempfile

        kw["tmpdir"] = tempfile.mkdtemp(prefix="trace_", dir=os.getcwd())
    res = run_bass_kernel_spmd(
        _get_nc(), in_maps, list(range(N_CORES)), trace=TRACE, **kw
    )
    _LAST_RESULT = res

    # ---- host combine (fp64) ----
    S = np.zeros(NR, dtype=np.float64)
    for c in range(N_CORES):
        rs = np.asarray(res.results[c]["rs"], np.float64)    # [128, 8]
        cs = np.asarray(res.results[c]["cs"], np.float64)    # [15, 512]
        base = BLK * c
        for bl in range(2):
            col0 = BLOCK_COL0[bl]
            for t in range(4):
                rows = (base + col0 + P * t + np.arange(P)) % NR
                S[rows] += rs[:, 4 * bl + t]
            for kk in BLOCK_CS[bl]:
                cols = (base + col0 + CH * kk + np.arange(CH)) % NR
                S[cols] += cs[CS_SLOT[(bl, kk)]]

    denom = S - E2
    partner = (np.arange(NR) + B) % NR
    pos = np.einsum(
        "ij,ij->i", z.astype(np.float64), z[partner].astype(np.float64)
    )
    loss = np.mean(np.log(denom)) - INV_T * np.mean(pos)
    return np.asarray(np.float32(loss))
